# revision 6
# baseline (speedup 1.0000x reference)
"""ClusterNorm1d kernel for Trainium2 (Bass/Tile), 8-core data parallel.

out[b,d,k] = sum_e Std_inv[k,d,e] * (x[b,e,k] - mu[e,k])

Strategy (v2, bf16 / transpose-free):
  - Shard batch B=8192 across 8 cores (1024 rows each).
  - Host packs x pre-transposed and pair-interleaved in bf16:
      xt[c, j, b] = x[b, e, j + 64*p]   with c = 2e + p
    so clusters (j, j+64) share one 128-deep contraction. Weights are the
    same block-diagonal pair panels as before:
      W[c=2e+pc, j, n=2d+pd] = S[j+64*pd, d, e] * (pc == pd)
  - Device work per pair j is then a single stationary-weight matmul
      psum[n, b] = sum_c W[c, j, n] * xt[c, j, b]
    (no on-device transpose at all), followed by a PSUM->SBUF drain fused
    with the -S@mu bias (per-partition scalar), alternating ACT / DVE.
  - Output returns transposed+packed [n', j, b] in bf16; the host unpacks
    to [B, D, K] f32. fp32 matmul costs 4 PE cycles/row vs 1 for bf16, so
    the bf16 datapath also removes the PE bottleneck (rel err ~1e-2 budget).
  - DMA: input stream on the SP HWDGE queue, output stream on the ACT
    HWDGE queue, 4 pairs (1 MiB) per transfer, 8 KiB contiguous per
    partition row.
"""

import numpy as np

B, D, K = 8192, 64, 128
N_CORES = 8
B_SHARD = B // N_CORES  # 1024
P = 128                 # SBUF partitions
NPAIR = K // 2          # 64 cluster pairs: (j, j+64)
SEG = 512               # matmul moving free-dim per PSUM bank

# DMA chunking (pairs per transfer). Packet size per partition row is
# 2*b_shard*csize bytes; bigger packets amortize the ~100ns/packet engine
# overhead, but the head (input) and tail (output) want small chunks so the
# pipeline starts/drains early.
IN_CHUNKS = [2, 2, 4, 8] + [16] * 3
OUT_CHUNKS = [16] * 3 + [8, 4, 2, 2]
W_CHUNKS = 4            # w panel DMA'd in 4 pair-range chunks

_cache = {}


def _bounds(chunks):
    out, s = [], 0
    for c in chunks:
        out.append((s, c))
        s += c
    return out


def _build_nc(b_shard):
    import concourse.tile as tile
    from concourse import bacc, mybir

    f32 = mybir.dt.float32
    bf16 = mybir.dt.bfloat16
    nc = bacc.Bacc("TRN2", target_bir_lowering=False)

    xt_d = nc.dram_tensor("xt", [P, NPAIR, b_shard], bf16, kind="ExternalInput")
    w_d = nc.dram_tensor("w", [P, NPAIR, P], bf16, kind="ExternalInput")
    nb_d = nc.dram_tensor("nbias", [P, NPAIR], f32, kind="ExternalInput")
    o_d = nc.dram_tensor("out", [P, NPAIR, b_shard], bf16, kind="ExternalOutput")

    seg = min(SEG, b_shard)
    nseg = b_shard // seg
    in_bounds = _bounds(IN_CHUNKS)
    out_bounds = _bounds(OUT_CHUNKS)

    with tile.TileContext(nc) as tc:
        with (
            tc.tile_pool(name="consts", bufs=1) as consts,
            tc.tile_pool(name="xin", bufs=2) as xin,
            tc.tile_pool(name="oout", bufs=2) as oout,
            tc.tile_pool(name="ps", bufs=3, space="PSUM") as psp,
        ):
            w_sb = consts.tile([P, NPAIR, P], bf16)
            nb_sb = consts.tile([P, NPAIR], f32)
            # First w chunk + bias ride the SP queue ahead of the x stream
            # (the SP DGE starts ~3us before ACT's); the rest of w rides the
            # ACT queue, which is otherwise idle until the first out chunk.
            wc = NPAIR // W_CHUNKS
            nc.sync.dma_start(out=w_sb[:, 0:wc, :], in_=w_d[:, 0:wc, :])
            nc.sync.dma_start(out=nb_sb, in_=nb_d[:])
            for q in range(1, W_CHUNKS):
                nc.scalar.dma_start(out=w_sb[:, q * wc:(q + 1) * wc, :],
                                    in_=w_d[:, q * wc:(q + 1) * wc, :])

            # Engine warm-ups: observe const semaphores once each.
            warm_ps = psp.tile([P, 2, seg], f32, tag="ps")
            nc.tensor.matmul(warm_ps[:, 0, 0:P], lhsT=w_sb[:, 0, :],
                             rhs=w_sb[:, 0, :])
            scratch = consts.tile([P, 2], f32)
            nc.scalar.copy(out=scratch[:, 0:1], in_=nb_sb[:, 0:1])
            nc.vector.tensor_copy(out=scratch[:, 1:2], in_=nb_sb[:, 0:1])

            in_it = iter(in_bounds)
            out_it = iter(out_bounds)
            xt = o_sb = None
            in_s = in_n = out_s = out_n = 0
            for j in range(NPAIR):
                if xt is None or j >= in_s + in_n:
                    in_s, in_n = next(in_it)
                    xt = xin.tile([P, in_n, b_shard], bf16, tag="xt")
                    nc.sync.dma_start(
                        out=xt, in_=xt_d[:, in_s:in_s + in_n, :])
                if o_sb is None or j >= out_s + out_n:
                    out_s, out_n = next(out_it)
                    o_sb = oout.tile([P, out_n, b_shard], bf16, tag="o")
                # both halves of pair j land in one 2-bank PSUM tile, then
                # drain in a single bias-fused op (alternating ACT/DVE)
                ps = psp.tile([P, nseg, seg], f32, tag="ps")
                for h in range(nseg):
                    nc.tensor.matmul(
                        ps[:, h, :], lhsT=w_sb[:, j, :],
                        rhs=xt[:, j - in_s, h * seg:(h + 1) * seg])
                dst = o_sb[:, j - out_s, :]
                src = ps.rearrange("p a b -> p (a b)")
                nbj = nb_sb[:, j:j + 1]
                if j % 2 == 0:
                    nc.scalar.add(dst, src, nbj)
                else:
                    nc.vector.tensor_scalar_add(dst, src, nbj)
                if j == out_s + out_n - 1:
                    nc.scalar.dma_start(
                        out=o_d[:, out_s:out_s + out_n, :], in_=o_sb)

    nc.compile()
    return nc


def _host_prep(mu_track, Std_inv_track):
    """Block-diagonal pair weights W[c=2e+pc, j, n=2d+pd] (bf16) and the
    negated per-partition bias nbias[n'=2d+p, j] = -(S@mu)[d, j+64p] (f32)."""
    import ml_dtypes

    S = np.ascontiguousarray(Std_inv_track, dtype=np.float32)
    mu = np.ascontiguousarray(mu_track, dtype=np.float32)

    W = np.zeros((2 * D, NPAIR, 2 * D), dtype=np.float32)
    W6 = W.reshape(D, 2, NPAIR, D, 2)                 # [e, pc, j, d, pd]
    S_r = S.reshape(2, NPAIR, D, D)                   # [pk, j, d, e]
    W6[:, 0, :, :, 0] = S_r[0].transpose(2, 0, 1)     # [e, j, d]
    W6[:, 1, :, :, 1] = S_r[1].transpose(2, 0, 1)

    bias_dk = np.einsum("kde,ek->dk", S, mu)          # [d, k], k = 64p + j
    nbias = -bias_dk.reshape(D, 2, NPAIR).reshape(2 * D, NPAIR)  # [n'=2d+p, j]
    return W.astype(ml_dtypes.bfloat16), np.ascontiguousarray(nbias)


def _pack_x(x, n_cores, b_shard):
    """x [n_cores*b_shard, D, K] f32 -> xt [n_cores, 128, NPAIR, b_shard] bf16
    with xt[core, 2e+p, j, b] = x[core*b_shard + b, e, j + 64p]."""
    import ml_dtypes

    xb = np.ascontiguousarray(x, dtype=np.float32).astype(ml_dtypes.bfloat16)
    xp = xb.reshape(n_cores, b_shard, D, 2, NPAIR)    # [core, b, e, p, j]
    xt = xp.transpose(0, 2, 3, 4, 1)                  # [core, e, p, j, b]
    return np.ascontiguousarray(xt).reshape(n_cores, P, NPAIR, b_shard)


def _unpack_out(oT, n_cores, b_shard):
    """oT [n_cores, 128, NPAIR, b_shard] bf16 -> out [n_cores*b_shard, D, K]
    f32 with out[b, d, j+64p] = oT[core, 2d+p, j, b]."""
    ov = oT.reshape(n_cores, D, 2, NPAIR, b_shard)    # [core, d, p, j, b]
    out = ov.transpose(0, 4, 1, 2, 3)                 # [core, b, d, p, j]
    return np.ascontiguousarray(out).reshape(
        n_cores * b_shard, D, K).astype(np.float32)


def kernel(x, mu_track, Std_inv_track):
    from concourse.bass_utils import run_bass_kernel_spmd

    xt = _pack_x(x, N_CORES, B_SHARD)
    W, nbias = _host_prep(mu_track, Std_inv_track)

    if "nc" not in _cache:
        _cache["nc"] = _build_nc(B_SHARD)
    nc = _cache["nc"]

    in_maps = []
    for i in range(N_CORES):
        in_maps.append({"xt": xt[i], "w": W, "nbias": nbias})
    res = run_bass_kernel_spmd(nc, in_maps, core_ids=list(range(N_CORES)))
    oT = np.stack([r["out"] for r in res.results], axis=0)
    return _unpack_out(oT, N_CORES, B_SHARD)


# revision 8
# speedup vs baseline: 1.0793x; 1.0793x over previous
"""ClusterNorm1d kernel for Trainium2 (Bass/Tile), 8-core data parallel.

out[b,d,k] = sum_e Std_inv[k,d,e] * (x[b,e,k] - mu[e,k])

Strategy (v2, bf16 / transpose-free):
  - Shard batch B=8192 across 8 cores (1024 rows each).
  - Host packs x pre-transposed and pair-interleaved in bf16:
      xt[c, j, b] = x[b, e, j + 64*p]   with c = 2e + p
    so clusters (j, j+64) share one 128-deep contraction. Weights are the
    same block-diagonal pair panels as before:
      W[c=2e+pc, j, n=2d+pd] = S[j+64*pd, d, e] * (pc == pd)
  - Device work per pair j is then a single stationary-weight matmul
      psum[n, b] = sum_c W[c, j, n] * xt[c, j, b]
    (no on-device transpose at all), followed by a PSUM->SBUF drain fused
    with the -S@mu bias (per-partition scalar), alternating ACT / DVE.
  - Output returns transposed+packed [n', j, b] in bf16; the host unpacks
    to [B, D, K] f32. fp32 matmul costs 4 PE cycles/row vs 1 for bf16, so
    the bf16 datapath also removes the PE bottleneck (rel err ~1e-2 budget).
  - DMA: input stream on the SP HWDGE queue, output stream on the ACT
    HWDGE queue, 4 pairs (1 MiB) per transfer, 8 KiB contiguous per
    partition row.
"""

import numpy as np

B, D, K = 8192, 64, 128
N_CORES = 8
B_SHARD = B // N_CORES  # 1024
P = 128                 # SBUF partitions
NPAIR = K // 2          # 64 cluster pairs: (j, j+64)
SEG = 512               # matmul moving free-dim per PSUM bank

# DMA chunking (pairs per transfer). Packet size per partition row is
# 2*b_shard*csize bytes; bigger packets amortize the ~100ns/packet engine
# overhead, but the head (input) and tail (output) want small chunks so the
# pipeline starts/drains early.
IN_CHUNKS = [2, 2, 4] + [8] * 7
OUT_CHUNKS = [8] * 7 + [4, 2, 2]
W_CHUNKS = 4            # w panel DMA'd in 4 pair-range chunks

_cache = {}


def _bounds(chunks):
    out, s = [], 0
    for c in chunks:
        out.append((s, c))
        s += c
    return out


def _build_nc(b_shard):
    import concourse.tile as tile
    from concourse import bacc, mybir

    f32 = mybir.dt.float32
    bf16 = mybir.dt.bfloat16
    nc = bacc.Bacc("TRN2", target_bir_lowering=False)

    xt_d = nc.dram_tensor("xt", [P, NPAIR, b_shard], bf16, kind="ExternalInput")
    w_d = nc.dram_tensor("w", [P, NPAIR, P], bf16, kind="ExternalInput")
    nb_d = nc.dram_tensor("nbias", [P, NPAIR], f32, kind="ExternalInput")
    o_d = nc.dram_tensor("out", [P, NPAIR, b_shard], bf16, kind="ExternalOutput")

    seg = min(SEG, b_shard)
    nseg = b_shard // seg
    in_bounds = _bounds(IN_CHUNKS)
    out_bounds = _bounds(OUT_CHUNKS)

    with tile.TileContext(nc) as tc:
        with (
            tc.tile_pool(name="consts", bufs=1) as consts,
            tc.tile_pool(name="xin", bufs=6) as xin,
            tc.tile_pool(name="oout", bufs=3) as oout,
            tc.tile_pool(name="ps", bufs=4, space="PSUM") as psp,
        ):
            w_sb = consts.tile([P, NPAIR, P], bf16)
            nb_sb = consts.tile([P, NPAIR], f32)
            # First w chunk + bias ride the SP queue ahead of the x stream
            # (the SP DGE starts ~3us before ACT's); the rest of w rides the
            # ACT queue, which is otherwise idle until the first out chunk.
            wc = NPAIR // W_CHUNKS
            nc.sync.dma_start(out=w_sb[:, 0:wc, :], in_=w_d[:, 0:wc, :])
            nc.sync.dma_start(out=nb_sb, in_=nb_d[:])
            for q in range(1, W_CHUNKS):
                nc.scalar.dma_start(out=w_sb[:, q * wc:(q + 1) * wc, :],
                                    in_=w_d[:, q * wc:(q + 1) * wc, :])

            # Engine warm-ups: observe const semaphores once each.
            warm_ps = psp.tile([P, 2, seg], f32, tag="ps")
            nc.tensor.matmul(warm_ps[:, 0, 0:P], lhsT=w_sb[:, 0, :],
                             rhs=w_sb[:, 0, :])
            scratch = consts.tile([P, 2], f32)
            nc.scalar.copy(out=scratch[:, 0:1], in_=nb_sb[:, 0:1])
            nc.vector.tensor_copy(out=scratch[:, 1:2], in_=nb_sb[:, 0:1])

            in_it = iter(in_bounds)
            out_it = iter(out_bounds)
            xt = o_sb = None
            in_s = in_n = out_s = out_n = 0
            for j in range(NPAIR):
                if xt is None or j >= in_s + in_n:
                    in_s, in_n = next(in_it)
                    xt = xin.tile([P, in_n, b_shard], bf16, tag="xt")
                    nc.sync.dma_start(
                        out=xt, in_=xt_d[:, in_s:in_s + in_n, :])
                if o_sb is None or j >= out_s + out_n:
                    out_s, out_n = next(out_it)
                    o_sb = oout.tile([P, out_n, b_shard], bf16, tag="o")
                # both halves of pair j land in one 2-bank PSUM tile, then
                # drain in a single bias-fused op (alternating ACT/DVE)
                ps = psp.tile([P, nseg, seg], f32, tag="ps")
                for h in range(nseg):
                    nc.tensor.matmul(
                        ps[:, h, :], lhsT=w_sb[:, j, :],
                        rhs=xt[:, j - in_s, h * seg:(h + 1) * seg])
                dst = o_sb[:, j - out_s, :]
                src = ps.rearrange("p a b -> p (a b)")
                nbj = nb_sb[:, j:j + 1]
                if j % 2 == 0:
                    nc.scalar.add(dst, src, nbj)
                else:
                    nc.vector.tensor_scalar_add(dst, src, nbj)
                if j == out_s + out_n - 1:
                    nc.scalar.dma_start(
                        out=o_d[:, out_s:out_s + out_n, :], in_=o_sb)

    nc.compile()
    return nc


def _host_prep(mu_track, Std_inv_track):
    """Block-diagonal pair weights W[c=2e+pc, j, n=2d+pd] (bf16) and the
    negated per-partition bias nbias[n'=2d+p, j] = -(S@mu)[d, j+64p] (f32)."""
    import ml_dtypes

    S = np.ascontiguousarray(Std_inv_track, dtype=np.float32)
    mu = np.ascontiguousarray(mu_track, dtype=np.float32)

    W = np.zeros((2 * D, NPAIR, 2 * D), dtype=np.float32)
    W6 = W.reshape(D, 2, NPAIR, D, 2)                 # [e, pc, j, d, pd]
    S_r = S.reshape(2, NPAIR, D, D)                   # [pk, j, d, e]
    W6[:, 0, :, :, 0] = S_r[0].transpose(2, 0, 1)     # [e, j, d]
    W6[:, 1, :, :, 1] = S_r[1].transpose(2, 0, 1)

    bias_dk = np.einsum("kde,ek->dk", S, mu)          # [d, k], k = 64p + j
    nbias = -bias_dk.reshape(D, 2, NPAIR).reshape(2 * D, NPAIR)  # [n'=2d+p, j]
    return W.astype(ml_dtypes.bfloat16), np.ascontiguousarray(nbias)


def _pack_x(x, n_cores, b_shard):
    """x [n_cores*b_shard, D, K] f32 -> xt [n_cores, 128, NPAIR, b_shard] bf16
    with xt[core, 2e+p, j, b] = x[core*b_shard + b, e, j + 64p]."""
    import ml_dtypes

    xb = np.ascontiguousarray(x, dtype=np.float32).astype(ml_dtypes.bfloat16)
    xp = xb.reshape(n_cores, b_shard, D, 2, NPAIR)    # [core, b, e, p, j]
    xt = xp.transpose(0, 2, 3, 4, 1)                  # [core, e, p, j, b]
    return np.ascontiguousarray(xt).reshape(n_cores, P, NPAIR, b_shard)


def _unpack_out(oT, n_cores, b_shard):
    """oT [n_cores, 128, NPAIR, b_shard] bf16 -> out [n_cores*b_shard, D, K]
    f32 with out[b, d, j+64p] = oT[core, 2d+p, j, b]."""
    ov = oT.reshape(n_cores, D, 2, NPAIR, b_shard)    # [core, d, p, j, b]
    out = ov.transpose(0, 4, 1, 2, 3)                 # [core, b, d, p, j]
    return np.ascontiguousarray(out).reshape(
        n_cores * b_shard, D, K).astype(np.float32)


def kernel(x, mu_track, Std_inv_track):
    from concourse.bass_utils import run_bass_kernel_spmd

    xt = _pack_x(x, N_CORES, B_SHARD)
    W, nbias = _host_prep(mu_track, Std_inv_track)

    if "nc" not in _cache:
        _cache["nc"] = _build_nc(B_SHARD)
    nc = _cache["nc"]

    in_maps = []
    for i in range(N_CORES):
        in_maps.append({"xt": xt[i], "w": W, "nbias": nbias})
    res = run_bass_kernel_spmd(nc, in_maps, core_ids=list(range(N_CORES)))
    oT = np.stack([r["out"] for r in res.results], axis=0)
    return _unpack_out(oT, N_CORES, B_SHARD)
